# revision 34
# baseline (speedup 1.0000x reference)
"""AutoCorrelation (factor=3) Trainium2 kernel, 8-core batch-parallel.

Math: the reference's corr tensor [B,L,2047] is only ever used through its
mean over L. That mean collapses to quadratic forms of the Gram matrix
M_b = q_b^T k_b (one [512,512] matmul per batch):
    Zbar[f] = c_f^T M c_f + s_f^T M s_f  + i (c_f^T M s_f - s_f^T M c_f)
mean_value = irfft(Zbar/L, 2047) is a tiny [513]->[2047] cos/sin matmul
(done on host), and the final roll-sum is a circular correlation done
spectrally with constant DFT matrices (all dense matmuls on device).

The f=512 Nyquist column is packed into the (always-zero) sin slot f=0 on
both device stages so every tensor is exactly 512 wide (4x128 tiles, all
even sizes -> fp32r-legal). NEFF1's U-stage uses four independent host
tables so the packed slots unpack cleanly:
    u1 = uc1*T1 + us1*T2   (col0: 1*T1[:,0])         -> Zr[0..511]
    u2 = uc2*T2 - us1*T1   (col0: c512*T2[:,0])      -> [Zr[512], Zi[1..511]]
where the T-stage sin table has col0 := cos(pi*d) so T2[:,0] = M c512.

NEFF1 (per core, batch b): N = k^T q; T1 = N^T tcT; T2 = N^T tsT;
    u1/u2 elementwise; Zr/Zi = partition-sum (gpsimd) -> zout [2,512]
Host: mean_value -> top-20 -> softmax weights w[b], batch-0 shifts ->
    per-batch spectral coefficient vectors a,b,c,d [512].
NEFF2 (per core): Vr/Vi = DFT_L(v) (Nyquist packed into Vi row 0);
    Hr = a*Vr + b*Vi; Hi = c*Vi + d*Vr; out = ICr^T Hr + ICs^T Hi.
"""
import math
import numpy as np

from contextlib import ExitStack
from concourse import bass, mybir, tile, bacc
from concourse.bass_utils import run_bass_kernel_spmd

B, L, D = 8, 1024, 512
NF = L // 2 + 1      # 513
T = 2 * L - 1        # 2047
K = int(3 * math.log(float(L)))  # 20
F32 = mybir.dt.float32

# matmul compute dtype: float32 (safe) or float32r (full-rate fp32 path)
MM_DT = mybir.dt.float32r

NCORES = 8
CORE_IDS = list(range(NCORES))

_cache = {}


# ---------------------------------------------------------------- tables
def _tables():
    """KER[j, t]: mean_value = G @ KER, where G[j] is the diagonal sum of
    N = k^T q at offset Delta = j - 512. Combines the d-axis DFT of G with
    the irfft-to-2047 of Zbar/L (both tiny, fused into one [1024, 2047]
    host matrix)."""
    if 'tables' in _cache:
        return _cache['tables']
    f = np.arange(NF)

    ang2 = 2 * np.pi * np.outer(f, np.arange(T)) / T   # [513, 2047]
    alpha = np.full(NF, 2.0); alpha[0] = 1.0
    C2 = alpha[:, None] * np.cos(ang2) / (T * L)
    S2 = -2.0 * np.sin(ang2) / (T * L); S2[0] = 0.0

    delta = np.arange(1024) - 512                      # [1024]
    angd = 2 * np.pi * np.outer(delta, f) / L          # [1024, 513]
    KER = np.cos(angd) @ C2 - np.sin(angd) @ S2        # [1024, 2047]

    tabs = dict(KER=KER)
    _cache['tables'] = tabs
    return tabs


# ---------------------------------------------------------------- NEFF 1
def build_neff1():
    """Zbar[f] = sum_Delta G[Delta] e^{-i 2pi f Delta / L} where
    G[Delta] = sum of the Delta-th diagonal of N = k^T q (Delta in
    [-511, 511]). Compute N on the PE, bounce it through a zero-padded
    DRAM buffer laid out [512 rows x 1536 cols] (zeros | N | zeros), and
    re-read with a skewed AP (partition stride = 1537 elements) so row p
    lands shifted by p: column sums of the two skewed views give the
    positive/negative diagonal sums directly. The DFT of G happens on
    the host (1024x2047 matmul, trivial)."""
    nc = bacc.Bacc(None, target_bir_lowering=False, debug=False)
    q_d = nc.declare_dram_parameter('q', [L, D], MM_DT, isOutput=False)
    k_d = nc.declare_dram_parameter('k', [L, D], MM_DT, isOutput=False)
    z_d = nc.declare_dram_parameter('zout', [2 * D, 512], MM_DT, isOutput=True)

    LT, DT = L // 128, D // 128        # 8, 4
    ROWW = 3 * 512                     # padded row width in the bounce buf
    SKEW = ROWW + 1

    with tile.TileContext(nc) as tc, ExitStack() as ctx:
        pool = ctx.enter_context(tc.tile_pool(name='sb', bufs=1))
        skp = ctx.enter_context(tc.tile_pool(name='sk', bufs=4))
        psum = ctx.enter_context(
            tc.tile_pool(name='ps', bufs=1, space=bass.MemorySpace.PSUM))
        dram = ctx.enter_context(tc.tile_pool(name='dr', bufs=1, space='DRAM'))

        # flat bounce buffer; extra tail so the [128,1537] windows exist
        n2f = dram.tile([D * ROWW + 2048], MM_DT)

        def rows(t, w=ROWW):
            # [128, w]-strided view of row block t of the bounce buffer
            return n2f[t * 128 * w: (t + 1) * 128 * w].rearrange(
                '(p c) -> p c', c=w)

        def skew(t, plus):
            start = t * 128 * SKEW + (512 if plus else 0)
            return n2f[start: start + 128 * SKEW].rearrange(
                '(p c) -> p c', c=SKEW)[:, 0:512]

        q_sb = pool.tile([128, LT, D], MM_DT)
        k_sb = pool.tile([128, LT, D], MM_DT)
        zero_f = pool.tile([128, 512], F32)
        nc.vector.memset(zero_f[:], 0.0)
        for i in range(LT):
            nc.sync.dma_start(q_sb[:, i, :], q_d[i * 128:(i + 1) * 128, :])
            nc.scalar.dma_start(k_sb[:, i, :], k_d[i * 128:(i + 1) * 128, :])

        zero_sb = pool.tile([128, 512], MM_DT)
        nc.vector.tensor_copy(zero_sb[:], zero_f[:])
        # zero the pad columns (left 512, right 512 of each row block) on
        # the scalar queue so the q/k input stream owns the sync queue
        for t in range(DT):
            nc.scalar.dma_start(rows(t)[:, 0:512], zero_sb[:])
            nc.scalar.dma_start(rows(t)[:, 1024:1536], zero_sb[:])

        # N[d2, d1] = sum_l k[l,d2] q[l,d1]; bounce rows to DRAM;
        # skew-read both diagonal halves; column-reduce via ones-matmul.
        pns = [psum.tile([128, D], F32, tag=f'pn{t2}', name=f'pn{t2}')
               for t2 in range(DT)]
        for lt in range(LT):
            for t2 in range(DT):
                nc.tensor.matmul(
                    pns[t2][:],
                    k_sb[:, lt, t2 * 128:(t2 + 1) * 128],
                    q_sb[:, lt, :],
                    start=(lt == 0), stop=(lt == LT - 1))
        # bounce N rows to DRAM, then ship both skewed diagonal views
        # straight DRAM->DRAM into the output; the host column-sums them.
        for t2 in range(DT):
            n_t = skp.tile([128, 512], MM_DT, tag='nt')
            nc.vector.tensor_copy(n_t[:], pns[t2][:])
            nc.sync.dma_start(rows(t2)[:, 512:1024], n_t[:])
        for t2 in range(DT):
            nc.scalar.dma_start(
                z_d[D + t2 * 128: D + (t2 + 1) * 128, :], skew(t2, True))
            nc.sync.dma_start(
                z_d[t2 * 128: (t2 + 1) * 128, :], skew(t2, False))

    nc.finalize()
    return nc


# ---------------------------------------------------------------- NEFF 2
def build_neff2():
    """out[l,d] = sum_m At[m,l] v[m,d] with At[m,l] = coef[(m-l) mod L]:
    the weighted roll-sum is a circulant matmul (one [1024,1024]@[1024,512]
    per batch), At built on host from the 20 softmax weights."""
    nc = bacc.Bacc(None, target_bir_lowering=False, debug=False)
    v_d = nc.declare_dram_parameter('v', [L, D], MM_DT, isOutput=False)
    at_d = nc.declare_dram_parameter('at', [L, L], MM_DT, isOutput=False)
    o_d = nc.declare_dram_parameter('out', [L, D], F32, isOutput=True)

    LT = L // 128                      # 8

    with tile.TileContext(nc) as tc, ExitStack() as ctx:
        pool = ctx.enter_context(tc.tile_pool(name='sb', bufs=1))
        outp = ctx.enter_context(tc.tile_pool(name='op', bufs=3))
        psum_o = ctx.enter_context(
            tc.tile_pool(name='pso', bufs=1, space=bass.MemorySpace.PSUM))

        v_sb = pool.tile([128, LT, D], MM_DT)
        at_sb = pool.tile([128, LT, L], MM_DT)
        for i in range(LT):
            nc.sync.dma_start(v_sb[:, i, :], v_d[i * 128:(i + 1) * 128, :])
            nc.scalar.dma_start(at_sb[:, i, :], at_d[i * 128:(i + 1) * 128, :])

        # out[l,d] = sum_m At[m,l] v[m,d]. mt-outer with all 8 PSUM
        # accumulation groups live: the PE gets 8 back-to-back matmuls per
        # arriving (At,v) tile pair and stays dense enough to hold the
        # high HAM p-state.
        pos = [psum_o.tile([128, D], F32, tag=f'po{lt}', name=f'po{lt}')
               for lt in range(LT)]
        for mt in range(LT):
            for lt in range(LT):
                nc.tensor.matmul(
                    pos[lt][:],
                    at_sb[:, mt, lt * 128:(lt + 1) * 128],
                    v_sb[:, mt, :],
                    start=(mt == 0), stop=(mt == LT - 1))
        for lt in range(LT):
            o_sb = outp.tile([128, D], F32)
            nc.vector.tensor_copy(o_sb[:], pos[lt][:])
            eng = nc.sync if lt % 2 == 0 else nc.scalar
            eng.dma_start(o_d[lt * 128:(lt + 1) * 128, :], o_sb[:])

    nc.finalize()
    return nc


# ---------------------------------------------------------------- driver
def _get_graphs():
    if 'nc1' not in _cache:
        _cache['nc1'] = build_neff1()
        _cache['nc2'] = build_neff2()
    return _cache['nc1'], _cache['nc2']


def kernel(queries, keys, values, _trace=False):
    tabs = _tables()
    nc1, nc2 = _get_graphs()
    q = np.ascontiguousarray(np.asarray(queries, np.float32))
    k = np.ascontiguousarray(np.asarray(keys, np.float32))
    v = np.ascontiguousarray(np.asarray(values, np.float32))

    in1 = [{'q': q[b], 'k': k[b]} for b in range(B)]
    r1 = run_bass_kernel_spmd(nc1, in1, core_ids=CORE_IDS, trace=_trace)
    z = np.stack([r1.results[b]['zout'] for b in range(B)])   # [B, 1024, 512]

    # g[j] = diagonal sum of N at Delta = j - 512 (rows 0:512 = negative
    # half, 512:1024 = positive half; device ships raw skewed views)
    g = z.reshape(B, 2, 4, 128, 512).sum(axis=(2, 3)).reshape(B, 1024)
    mean_value = g @ tabs['KER']                              # [B, T]
    ind = np.argsort(-mean_value, axis=-1, kind='stable')[:, :K]
    val = np.take_along_axis(mean_value, ind, axis=-1)
    e = np.exp(val - val.max(-1, keepdims=True))
    w = e / e.sum(-1, keepdims=True)                          # [B, K]
    shifts = ind[0]                                           # [K]

    # circulant build: coef[j] = sum_k w[b,k] [j == s_k mod L];
    # At[m,l] = coef[(m-l) mod L] via an as_strided view of 3x-tiled coef
    sh = shifts % L
    ats = []
    for b in range(B):
        coef = np.zeros(L, np.float32)
        np.add.at(coef, sh, w[b].astype(np.float32))
        coef3 = np.concatenate([coef, coef, coef])
        view = np.lib.stride_tricks.as_strided(
            coef3[L:], shape=(L, L), strides=(4, -4))
        ats.append(np.ascontiguousarray(view))

    in2 = [{'v': v[b], 'at': ats[b]} for b in range(B)]
    r2 = run_bass_kernel_spmd(nc2, in2, core_ids=CORE_IDS, trace=_trace)
    out = np.stack([r2.results[b]['out'] for b in range(B)])  # [B, L, D]

    kernel._last_exec_ns = (
        (r1.exec_time_ns or 0) + (r2.exec_time_ns or 0)
        if (r1.exec_time_ns or r2.exec_time_ns) else None)
    kernel._last_results = (r1, r2)
    return out.astype(np.float32)


# revision 38
# speedup vs baseline: 1.0777x; 1.0777x over previous
"""AutoCorrelation (factor=3) Trainium2 kernel, 8-core batch-parallel.

Math: the reference's corr tensor [B,L,2047] is only ever used through its
mean over L. That mean collapses to quadratic forms of the Gram matrix
M_b = q_b^T k_b (one [512,512] matmul per batch):
    Zbar[f] = c_f^T M c_f + s_f^T M s_f  + i (c_f^T M s_f - s_f^T M c_f)
mean_value = irfft(Zbar/L, 2047) is a tiny [513]->[2047] cos/sin matmul
(done on host), and the final roll-sum is a circular correlation done
spectrally with constant DFT matrices (all dense matmuls on device).

The f=512 Nyquist column is packed into the (always-zero) sin slot f=0 on
both device stages so every tensor is exactly 512 wide (4x128 tiles, all
even sizes -> fp32r-legal). NEFF1's U-stage uses four independent host
tables so the packed slots unpack cleanly:
    u1 = uc1*T1 + us1*T2   (col0: 1*T1[:,0])         -> Zr[0..511]
    u2 = uc2*T2 - us1*T1   (col0: c512*T2[:,0])      -> [Zr[512], Zi[1..511]]
where the T-stage sin table has col0 := cos(pi*d) so T2[:,0] = M c512.

NEFF1 (per core, batch b): N = k^T q; T1 = N^T tcT; T2 = N^T tsT;
    u1/u2 elementwise; Zr/Zi = partition-sum (gpsimd) -> zout [2,512]
Host: mean_value -> top-20 -> softmax weights w[b], batch-0 shifts ->
    per-batch spectral coefficient vectors a,b,c,d [512].
NEFF2 (per core): Vr/Vi = DFT_L(v) (Nyquist packed into Vi row 0);
    Hr = a*Vr + b*Vi; Hi = c*Vi + d*Vr; out = ICr^T Hr + ICs^T Hi.
"""
import math
import numpy as np

from contextlib import ExitStack
from concourse import bass, mybir, tile, bacc
from concourse.bass_utils import run_bass_kernel_spmd

B, L, D = 8, 1024, 512
NF = L // 2 + 1      # 513
T = 2 * L - 1        # 2047
K = int(3 * math.log(float(L)))  # 20
F32 = mybir.dt.float32

# matmul compute dtype: float32 (safe) or float32r (full-rate fp32 path)
MM_DT = mybir.dt.float32r

NCORES = 8
CORE_IDS = list(range(NCORES))

_cache = {}


# ---------------------------------------------------------------- tables
def _tables():
    """KER[j, t]: mean_value = G @ KER, where G[j] is the diagonal sum of
    N = k^T q at offset Delta = j - 512. Combines the d-axis DFT of G with
    the irfft-to-2047 of Zbar/L (both tiny, fused into one [1024, 2047]
    host matrix)."""
    if 'tables' in _cache:
        return _cache['tables']
    f = np.arange(NF)

    ang2 = 2 * np.pi * np.outer(f, np.arange(T)) / T   # [513, 2047]
    alpha = np.full(NF, 2.0); alpha[0] = 1.0
    C2 = alpha[:, None] * np.cos(ang2) / (T * L)
    S2 = -2.0 * np.sin(ang2) / (T * L); S2[0] = 0.0

    delta = np.arange(1024) - 512                      # [1024]
    angd = 2 * np.pi * np.outer(delta, f) / L          # [1024, 513]
    KER = np.cos(angd) @ C2 - np.sin(angd) @ S2        # [1024, 2047]

    tabs = dict(KER=np.ascontiguousarray(KER, np.float32))
    _cache['tables'] = tabs
    return tabs


# ---------------------------------------------------------------- NEFF 1
def build_neff1():
    """Zbar[f] = sum_Delta G[Delta] e^{-i 2pi f Delta / L} where
    G[Delta] = sum of the Delta-th diagonal of N = k^T q (Delta in
    [-511, 511]). Compute N on the PE, bounce it through a zero-padded
    DRAM buffer laid out [512 rows x 1536 cols] (zeros | N | zeros), and
    re-read with a skewed AP (partition stride = 1537 elements) so row p
    lands shifted by p: column sums of the two skewed views give the
    positive/negative diagonal sums directly. The DFT of G happens on
    the host (1024x2047 matmul, trivial)."""
    nc = bacc.Bacc(None, target_bir_lowering=False, debug=False)
    q_d = nc.declare_dram_parameter('q', [L, D], MM_DT, isOutput=False)
    k_d = nc.declare_dram_parameter('k', [L, D], MM_DT, isOutput=False)
    z_d = nc.declare_dram_parameter('zout', [2 * D, 512], MM_DT, isOutput=True)

    LT, DT = L // 128, D // 128        # 8, 4
    ROWW = 3 * 512                     # padded row width in the bounce buf
    SKEW = ROWW + 1

    with tile.TileContext(nc) as tc, ExitStack() as ctx:
        pool = ctx.enter_context(tc.tile_pool(name='sb', bufs=1))
        skp = ctx.enter_context(tc.tile_pool(name='sk', bufs=4))
        psum = ctx.enter_context(
            tc.tile_pool(name='ps', bufs=1, space=bass.MemorySpace.PSUM))
        dram = ctx.enter_context(tc.tile_pool(name='dr', bufs=1, space='DRAM'))

        # flat bounce buffer; extra tail so the [128,1537] windows exist
        n2f = dram.tile([D * ROWW + 2048], MM_DT)

        def rows(t, w=ROWW):
            # [128, w]-strided view of row block t of the bounce buffer
            return n2f[t * 128 * w: (t + 1) * 128 * w].rearrange(
                '(p c) -> p c', c=w)

        def skew(t, plus):
            start = t * 128 * SKEW + (512 if plus else 0)
            return n2f[start: start + 128 * SKEW].rearrange(
                '(p c) -> p c', c=SKEW)[:, 0:512]

        q_sb = pool.tile([128, LT, D], MM_DT)
        k_sb = pool.tile([128, LT, D], MM_DT)
        zero_f = pool.tile([128, 512], F32)
        nc.vector.memset(zero_f[:], 0.0)
        for i in range(LT):
            nc.sync.dma_start(q_sb[:, i, :], q_d[i * 128:(i + 1) * 128, :])
            nc.scalar.dma_start(k_sb[:, i, :], k_d[i * 128:(i + 1) * 128, :])

        zero_sb = pool.tile([128, 512], MM_DT)
        nc.vector.tensor_copy(zero_sb[:], zero_f[:])
        # zero the pad columns (left 512, right 512 of each row block) on
        # the scalar queue so the q/k input stream owns the sync queue
        for t in range(DT):
            nc.scalar.dma_start(rows(t)[:, 0:512], zero_sb[:])
            nc.scalar.dma_start(rows(t)[:, 1024:1536], zero_sb[:])

        # N[d2, d1] = sum_l k[l,d2] q[l,d1]; bounce rows to DRAM;
        # skew-read both skewed diagonal halves straight to the output.
        pns = [psum.tile([128, D], F32, tag=f'pn{t2}', name=f'pn{t2}')
               for t2 in range(DT)]
        for lt in range(LT):
            for t2 in range(DT):
                nc.tensor.matmul(
                    pns[t2][:],
                    k_sb[:, lt, t2 * 128:(t2 + 1) * 128],
                    q_sb[:, lt, :],
                    start=(lt == 0), stop=(lt == LT - 1))
        # bounce N rows to DRAM, then ship both skewed diagonal views
        # straight DRAM->DRAM into the output; the host column-sums them.
        for t2 in range(DT):
            n_t = skp.tile([128, 512], MM_DT, tag='nt')
            nc.vector.tensor_copy(n_t[:], pns[t2][:])
            nc.sync.dma_start(rows(t2)[:, 512:1024], n_t[:])
        for t2 in range(DT):
            nc.scalar.dma_start(
                z_d[D + t2 * 128: D + (t2 + 1) * 128, :], skew(t2, True))
            nc.sync.dma_start(
                z_d[t2 * 128: (t2 + 1) * 128, :], skew(t2, False))

    nc.finalize()
    return nc


# ---------------------------------------------------------------- NEFF 2
def build_neff2():
    """out[l,d] = sum_m At[m,l] v[m,d] with At[m,l] = coef[(m-l) mod L]:
    the weighted roll-sum is a circulant matmul (one [1024,1024]@[1024,512]
    per batch), At built on host from the 20 softmax weights."""
    nc = bacc.Bacc(None, target_bir_lowering=False, debug=False)
    v_d = nc.declare_dram_parameter('v', [L, D], MM_DT, isOutput=False)
    at_d = nc.declare_dram_parameter('at', [L, L], MM_DT, isOutput=False)
    o_d = nc.declare_dram_parameter('out', [L, D], F32, isOutput=True)

    LT = L // 128                      # 8

    with tile.TileContext(nc) as tc, ExitStack() as ctx:
        pool = ctx.enter_context(tc.tile_pool(name='sb', bufs=1))
        outp = ctx.enter_context(tc.tile_pool(name='op', bufs=3))
        psum_o = ctx.enter_context(
            tc.tile_pool(name='pso', bufs=1, space=bass.MemorySpace.PSUM))

        v_sb = pool.tile([128, LT, D], MM_DT)
        at_sb = pool.tile([128, LT, L], MM_DT)
        for i in range(LT):
            nc.sync.dma_start(v_sb[:, i, :], v_d[i * 128:(i + 1) * 128, :])
            nc.scalar.dma_start(at_sb[:, i, :], at_d[i * 128:(i + 1) * 128, :])

        # out[l,d] = sum_m At[m,l] v[m,d]. mt-outer with all 8 PSUM
        # accumulation groups live: the PE gets 8 back-to-back matmuls per
        # arriving (At,v) tile pair and stays dense enough to hold the
        # high HAM p-state.
        pos = [psum_o.tile([128, D], F32, tag=f'po{lt}', name=f'po{lt}')
               for lt in range(LT)]
        for mt in range(LT):
            for lt in range(LT):
                nc.tensor.matmul(
                    pos[lt][:],
                    at_sb[:, mt, lt * 128:(lt + 1) * 128],
                    v_sb[:, mt, :],
                    start=(mt == 0), stop=(mt == LT - 1))
        for lt in range(LT):
            o_sb = outp.tile([128, D], F32)
            nc.vector.tensor_copy(o_sb[:], pos[lt][:])
            eng = nc.sync if lt % 2 == 0 else nc.scalar
            eng.dma_start(o_d[lt * 128:(lt + 1) * 128, :], o_sb[:])

    nc.finalize()
    return nc


# ---------------------------------------------------------------- driver
def _get_graphs():
    if 'nc1' not in _cache:
        _cache['nc1'] = build_neff1()
        _cache['nc2'] = build_neff2()
    return _cache['nc1'], _cache['nc2']


def kernel(queries, keys, values, _trace=False):
    tabs = _tables()
    nc1, nc2 = _get_graphs()
    q = np.ascontiguousarray(np.asarray(queries, np.float32))
    k = np.ascontiguousarray(np.asarray(keys, np.float32))
    v = np.ascontiguousarray(np.asarray(values, np.float32))

    in1 = [{'q': q[b], 'k': k[b]} for b in range(B)]
    r1 = run_bass_kernel_spmd(nc1, in1, core_ids=CORE_IDS, trace=_trace)
    z = np.stack([r1.results[b]['zout'] for b in range(B)])   # [B, 1024, 512]

    # g[j] = diagonal sum of N at Delta = j - 512 (rows 0:512 = negative
    # half, 512:1024 = positive half; device ships raw skewed views)
    g = z.reshape(B, 2, 4, 128, 512).sum(axis=(2, 3)).reshape(B, 1024)
    mean_value = g.astype(np.float32) @ tabs['KER']           # [B, T]
    ind = np.argsort(-mean_value, axis=-1, kind='stable')[:, :K]
    val = np.take_along_axis(mean_value, ind, axis=-1)
    e = np.exp(val - val.max(-1, keepdims=True))
    w = e / e.sum(-1, keepdims=True)                          # [B, K]
    shifts = ind[0]                                           # [K]

    # circulant build: At[m,l] = coef[(m-l) mod L] where coef is the
    # scatter of the 20 softmax weights at shifts mod L. Only <=20
    # diagonals are nonzero: write those into a cached zero buffer
    # (clearing the previous call's diagonals first).
    sh = shifts % L
    res = np.unique(sh)
    cols = np.arange(L)
    if 'at8' not in _cache:
        _cache['at8'] = np.zeros((B, L, L), np.float32)
        _cache['at_res'] = None
    at8 = _cache['at8']
    if _cache['at_res'] is not None:
        rr = (cols[None, :] + _cache['at_res'][:, None]) % L
        for b in range(B):
            at8[b][rr, cols[None, :]] = 0.0
    rows_i = (cols[None, :] + res[:, None]) % L              # [R, L]
    for b in range(B):
        coef = np.zeros(L, np.float32)
        np.add.at(coef, sh, w[b].astype(np.float32))
        at8[b][rows_i, cols[None, :]] = coef[res][:, None]
    _cache['at_res'] = res

    in2 = [{'v': v[b], 'at': at8[b]} for b in range(B)]
    r2 = run_bass_kernel_spmd(nc2, in2, core_ids=CORE_IDS, trace=_trace)
    out = np.stack([r2.results[b]['out'] for b in range(B)])  # [B, L, D]

    kernel._last_exec_ns = (
        (r1.exec_time_ns or 0) + (r2.exec_time_ns or 0)
        if (r1.exec_time_ns or r2.exec_time_ns) else None)
    kernel._last_results = (r1, r2)
    return out.astype(np.float32)
